# revision 4
# baseline (speedup 1.0000x reference)
"""Luong 'general' attention (B=16, S=8192, H=1024) on 8 Trainium2 cores.

Data-parallel over batch: each of the 8 cores processes 2 batches.
Per core / per batch:
  tq    = W_attn @ q[b]                    (DVE fused mul+reduce over W rows)
  score = (K[b] @ tq) / 32, masked softmax (DVE STT + ACT exp + partition reduce)
  ctx   = attn @ V[b]                      (PE matmuls, contraction over s on
                                            the partition axis -> no transposes)

Layout: s = p*64 + i  (p = SBUF partition, i = column).  Every DMA is then
contiguous per partition (4KB+ runs), and the attn column for tile i is the
matmul's stationary operand lhsT[:, i].
"""

import os as _os
import sys
from contextlib import ExitStack

import numpy as np

B, S, H = 16, 8192, 1024
NCORES = 8
B_LOC = B // NCORES          # 2 batches per core
P = 128                      # SBUF partitions
T_I = 4                      # i-columns per K/V DMA tile ([128, 4, 1024] = 2MiB fp32)
HC = H // P                  # 8 W-rows per partition
INV_SCALE = 1.0 / 32.0       # 1/sqrt(H)

# "float32" (exact) or "bfloat16" (half the DMA traffic for K/V/tq/attn dots)
KV_DTYPE = _os.environ.get("LUONG_KV_DTYPE", "bfloat16")
# partition-axis reduction: "gpsimd" (partition_all_reduce) or "pe" (transpose)
PREDUCE = _os.environ.get("LUONG_PREDUCE", "gpsimd")


def _mods():
    for p in ("/opt/trn_rl_repo", "/root/.axon_site/_ro/trn_rl_repo"):
        if p not in sys.path:
            sys.path.append(p)
    import concourse.bass as bass
    import concourse.tile as tile
    from concourse import bacc, bass_isa, mybir
    from concourse.bass_utils import run_bass_kernel_spmd

    return bass, tile, mybir, bacc, bass_isa, run_bass_kernel_spmd


def build(s=S, kv_dtype=KV_DTYPE, t_i=T_I, preduce=PREDUCE):
    """Build + compile the per-core Bass program (SPMD, same program all cores)."""
    bass, tile, mybir, bacc, bass_isa, _ = _mods()

    ni = s // P
    nt = ni // t_i
    f32 = mybir.dt.float32
    kv_dt = mybir.dt.bfloat16 if kv_dtype == "bfloat16" else f32
    mult = mybir.AluOpType.mult
    AX = mybir.AxisListType.X
    Exp = mybir.ActivationFunctionType.Exp

    nc = bacc.Bacc(
        "TRN2",
        target_bir_lowering=False,
        debug=False,
        num_devices=NCORES,
    )

    q_d = nc.dram_tensor("query_in", [B_LOC, H], f32, kind="ExternalInput").ap()
    k_d = nc.dram_tensor("keys_in", [B_LOC, s, H], kv_dt, kind="ExternalInput").ap()
    v_d = nc.dram_tensor("values_in", [B_LOC, s, H], kv_dt, kind="ExternalInput").ap()
    m_d = nc.dram_tensor("mask_in", [B_LOC, s], f32, kind="ExternalInput").ap()
    w_d = nc.dram_tensor("w_in", [H, H], f32, kind="ExternalInput").ap()
    ctx_d = nc.dram_tensor("context_out", [B_LOC, H], f32, kind="ExternalOutput").ap()
    attn_d = nc.dram_tensor("attn_out", [B_LOC, s], f32, kind="ExternalOutput").ap()
    tq_d = nc.dram_tensor("tq_scratch", [B_LOC, H], f32).ap()

    def bcast_row(row_ap):
        """[N] DRAM row -> [128, N] partition-broadcast access pattern."""
        return bass.AP(
            tensor=row_ap.tensor,
            offset=row_ap.offset,
            ap=[[0, P]] + [list(x) for x in row_ap.ap],
        )

    def fused_dot(out, in0, in1, scale, accum_out):
        """accum_out[p] = sum_f in0[p,f]*in1[p,f]*scale, via one DVE op."""
        nc.vector.scalar_tensor_tensor(
            out=out, in0=in0, scalar=float(scale), in1=in1,
            op0=mult, op1=mult, accum_out=accum_out,
        )

    with tile.TileContext(nc) as tc, ExitStack() as ctx:
        kv = ctx.enter_context(tc.tile_pool(name="kv", bufs=6))
        wp = ctx.enter_context(tc.tile_pool(name="wp", bufs=2))
        sm = ctx.enter_context(tc.tile_pool(name="sm", bufs=2))
        ones = ctx.enter_context(tc.tile_pool(name="ones", bufs=1))
        ps = ctx.enter_context(tc.tile_pool(name="ps", bufs=4, space="PSUM"))
        psb = ctx.enter_context(tc.tile_pool(name="psb", bufs=4, space="PSUM"))

        if preduce == "pe":
            from concourse.masks import make_identity

            ident = ones.tile([P, P], f32, tag="ident")
            make_identity(nc, ident)
            ones_neg = ones.tile([1, P], f32, tag="onesn")
            nc.vector.memset(ones_neg, -1.0)
            ones_pos = ones.tile([1, P], f32, tag="onesp")
            nc.vector.memset(ones_pos, 1.0)

        def part_reduce_bcast(rowstat, op_name):
            """[128,1] per-partition stat -> [128,1] global stat on all partitions.

            Returns (ap, negated) where ap holds the reduced value broadcast to
            all partitions; negated=True means ap holds -stat.
            """
            if preduce == "gpsimd":
                g = sm.tile([P, 1], f32, tag=f"g{op_name}")
                rop = bass_isa.ReduceOp.max if op_name == "max" else bass_isa.ReduceOp.add
                nc.gpsimd.partition_all_reduce(g, rowstat, channels=P, reduce_op=rop)
                return g, False
            # PE path: transpose [128,1] -> [1,128], reduce on free dim,
            # broadcast back with a rank-1 ones matmul (negated ones for max,
            # so the result can feed activation bias directly).
            tr = psb.tile([1, P], f32, tag="tr")
            nc.tensor.transpose(tr, rowstat, ident[:, 0:1])
            gsb = sm.tile([1, 1], f32, tag=f"gsb{op_name}")
            if op_name == "max":
                nc.vector.reduce_max(out=gsb, in_=tr, axis=AX)
            else:
                nc.vector.reduce_sum(out=gsb, in_=tr, axis=AX)
                inv = sm.tile([1, 1], f32, tag="invs")
                nc.vector.reciprocal(inv, gsb)
                gsb = inv
            bc = psb.tile([P, 1], f32, tag=f"bc{op_name}")
            lhs = ones_neg if op_name == "max" else ones_pos
            nc.tensor.matmul(bc, lhsT=lhs, rhs=gsb, start=True, stop=True)
            return bc, op_name == "max"

        for b in range(B_LOC):
            # ---------------- tq[b] = W_attn @ q[b] ----------------
            q_bc = sm.tile([P, H], f32, tag="qbc")
            nc.gpsimd.dma_start(out=q_bc, in_=bcast_row(q_d[b]))
            w_view = w_d.rearrange("(p c) h -> p c h", p=P)  # row p*HC+c at part p
            tq_sb = sm.tile([P, HC], f32, tag="tqsb")
            prod_w = sm.tile([P, H], f32, tag="prodw")
            for half in range(2):
                wt = wp.tile([P, HC // 2, H], f32, tag="w")
                nc.sync.dma_start(
                    out=wt, in_=w_view[:, half * (HC // 2) : (half + 1) * (HC // 2), :]
                )
                for j in range(HC // 2):
                    c = half * (HC // 2) + j
                    fused_dot(prod_w, wt[:, j, :], q_bc, 1.0, tq_sb[:, c : c + 1])
            # partition-major [128, 8] -> flat [1024] -> broadcast [128, 1024]
            nc.sync.dma_start(out=tq_d[b].rearrange("(p c) -> p c", p=P), in_=tq_sb)
            tq_bc = sm.tile([P, H], f32, tag="tqbc")
            nc.gpsimd.dma_start(out=tq_bc, in_=bcast_row(tq_d[b]))
            if kv_dt != f32:
                tq_kv = sm.tile([P, H], kv_dt, tag="tqkv")
                nc.vector.tensor_copy(out=tq_kv, in_=tq_bc)
            else:
                tq_kv = tq_bc

            # ---------------- scores = (K[b] @ tq) / 32 ----------------
            k_view = k_d[b].rearrange("(p i) h -> p i h", p=P)
            scores = sm.tile([P, ni], f32, tag="scores")
            prod_kv = sm.tile([P, H], kv_dt, tag="prodkv")
            for t in range(nt):
                kt = kv.tile([P, t_i, H], kv_dt, tag="kv")
                nc.sync.dma_start(out=kt, in_=k_view[:, t * t_i : (t + 1) * t_i, :])
                for j in range(t_i):
                    i_abs = t * t_i + j
                    fused_dot(
                        prod_kv, kt[:, j, :], tq_kv, INV_SCALE,
                        scores[:, i_abs : i_abs + 1],
                    )

            # ---------------- masked softmax over all of scores ----------------
            mask_f = sm.tile([P, ni], f32, tag="maskf")
            nc.sync.dma_start(out=mask_f, in_=m_d[b].rearrange("(p i) -> p i", p=P))

            rowmax = sm.tile([P, 1], f32, tag="rowmax")
            nc.vector.reduce_max(out=rowmax, in_=scores, axis=AX)
            gmax, negated = part_reduce_bcast(rowmax, "max")
            if not negated:
                negmax = sm.tile([P, 1], f32, tag="negmax")
                nc.vector.tensor_scalar_mul(negmax, gmax, -1.0)
            else:
                negmax = gmax

            # p_raw = exp(scores - gmax); p = p_raw * mask with fused row-sums
            p_raw = sm.tile([P, ni], f32, tag="praw")
            nc.scalar.activation(
                out=p_raw, in_=scores, func=Exp, bias=negmax, scale=1.0
            )
            p_m = sm.tile([P, ni], f32, tag="pm")
            rowsum = sm.tile([P, 1], f32, tag="rowsum")
            fused_dot(p_m, p_raw, mask_f, 1.0, rowsum)
            ginv, _ = part_reduce_bcast(rowsum, "sum")
            if preduce == "gpsimd":
                inv = sm.tile([P, 1], f32, tag="inv")
                nc.vector.reciprocal(inv, ginv)
            else:
                inv = ginv  # already reciprocal'd + broadcast
            attn_f = sm.tile([P, ni], f32, tag="attnf")
            nc.vector.tensor_scalar_mul(attn_f, p_m, inv)
            nc.sync.dma_start(
                out=attn_d[b].rearrange("(p i) -> p i", p=P), in_=attn_f
            )
            if kv_dt != f32:
                attn_mm = sm.tile([P, ni], kv_dt, tag="attnkv")
                nc.vector.tensor_copy(out=attn_mm, in_=attn_f)
            else:
                attn_mm = attn_f

            # ---------------- context = attn @ V[b] ----------------
            v_view = v_d[b].rearrange("(p i) h -> p i h", p=P)
            ps_lo = ps.tile([1, 512], f32, tag="ps")
            ps_hi = ps.tile([1, 512], f32, tag="ps")
            for t in range(nt):
                vt = kv.tile([P, t_i, H], kv_dt, tag="kv")
                nc.sync.dma_start(out=vt, in_=v_view[:, t * t_i : (t + 1) * t_i, :])
                for j in range(t_i):
                    i_abs = t * t_i + j
                    first = i_abs == 0
                    last = i_abs == ni - 1
                    nc.tensor.matmul(
                        ps_lo,
                        lhsT=attn_mm[:, i_abs : i_abs + 1],
                        rhs=vt[:, j, 0:512],
                        start=first,
                        stop=last,
                    )
                    nc.tensor.matmul(
                        ps_hi,
                        lhsT=attn_mm[:, i_abs : i_abs + 1],
                        rhs=vt[:, j, 512:1024],
                        start=first,
                        stop=last,
                    )
            ctx_sb = sm.tile([1, H], f32, tag="ctxsb")
            nc.vector.tensor_copy(out=ctx_sb[0:1, 0:512], in_=ps_lo)
            nc.vector.tensor_copy(out=ctx_sb[0:1, 512:1024], in_=ps_hi)
            nc.sync.dma_start(out=ctx_d[b], in_=ctx_sb[0:1, :])

    nc.compile()
    return nc


_CACHE = {}


def _get_nc():
    key = (S, KV_DTYPE, T_I, PREDUCE)
    if key not in _CACHE:
        _CACHE[key] = build()
    return _CACHE[key]


def kernel(query, keys, values, mask, W_attn):
    _, _, _, _, _, run_bass_kernel_spmd = _mods()
    import ml_dtypes

    kv_np = ml_dtypes.bfloat16 if KV_DTYPE == "bfloat16" else np.float32

    query = np.asarray(query, dtype=np.float32)
    keys = np.asarray(keys)
    values = np.asarray(values)
    mask_f = np.asarray(mask, dtype=np.float32)
    W_attn = np.asarray(W_attn, dtype=np.float32)

    nc = _get_nc()
    in_maps = []
    for m in range(NCORES):
        sl = slice(B_LOC * m, B_LOC * (m + 1))
        in_maps.append(
            {
                "query_in": np.ascontiguousarray(query[sl]),
                "keys_in": np.ascontiguousarray(keys[sl]).astype(kv_np),
                "values_in": np.ascontiguousarray(values[sl]).astype(kv_np),
                "mask_in": np.ascontiguousarray(mask_f[sl]),
                "w_in": W_attn,
            }
        )
    res = run_bass_kernel_spmd(nc, in_maps, list(range(NCORES))).results
    context = np.concatenate([r["context_out"] for r in res], axis=0)
    attn = np.concatenate([r["attn_out"] for r in res], axis=0)
    return context, attn


# revision 5
# speedup vs baseline: 1.2326x; 1.2326x over previous
"""Luong 'general' attention (B=16, S=8192, H=1024) on 8 Trainium2 cores.

Data-parallel over batch: each of the 8 cores processes 2 batches.
Per core / per batch:
  tq    = W_attn @ q[b]              (DVE fused mul+reduce over W rows)
  p_i   = exp((K[b,i] @ tq)/32 + maskterm)   per tile, no max-stabilization
          (logits are bounded ~|8|, so exp is fp32-safe; masked entries get
          -1e9 -> exp underflows to exactly 0, matching the reference)
  ctx   = (p @ V[b]) * (1/sum p)     (PE matmuls accumulate unnormalized p;
                                      one final scale by the softmax sum)

Dropping the max subtraction removes the K-phase -> softmax -> V-phase
barrier: each tile's exp + V matmuls fire as soon as its scores land, so the
K and V DMA streams interleave continuously and DMA stays saturated.

Layout: s = p*64 + i  (p = SBUF partition, i = column).  Every DMA is then
contiguous per partition (16KB runs), and the p-column for tile i is the
matmul's stationary operand lhsT[:, i] (contraction over s = partition axis,
so no transposes anywhere).
"""

import os as _os
import sys
from contextlib import ExitStack

import numpy as np

B, S, H = 16, 8192, 1024
NCORES = 8
B_LOC = B // NCORES          # 2 batches per core
P = 128                      # SBUF partitions
T_I = 4                      # i-columns per K/V DMA tile ([128, 4, 1024] = 2MiB fp32)
HC = H // P                  # 8 W-rows per partition
INV_SCALE = 1.0 / 32.0       # 1/sqrt(H)
BIG = 1.0e9

# dtypes for the streamed K / V tensors: "float32" (exact) or "bfloat16"
# (halves that tensor's DMA traffic; adds ~1e-3 relative error)
K_DTYPE = _os.environ.get("LUONG_K_DTYPE", "float32")
V_DTYPE = _os.environ.get("LUONG_V_DTYPE", "float32")
# partition-axis reduction: "gpsimd" (partition_all_reduce) or "pe" (transpose)
PREDUCE = _os.environ.get("LUONG_PREDUCE", "gpsimd")


def _mods():
    for p in ("/opt/trn_rl_repo", "/root/.axon_site/_ro/trn_rl_repo"):
        if p not in sys.path:
            sys.path.append(p)
    import concourse.bass as bass
    import concourse.tile as tile
    from concourse import bacc, bass_isa, mybir
    from concourse.bass_utils import run_bass_kernel_spmd

    return bass, tile, mybir, bacc, bass_isa, run_bass_kernel_spmd


def build(s=S, k_dtype=K_DTYPE, v_dtype=V_DTYPE, t_i=T_I, preduce=PREDUCE):
    """Build + compile the per-core Bass program (SPMD, same program all cores)."""
    bass, tile, mybir, bacc, bass_isa, _ = _mods()

    ni = s // P
    nt = ni // t_i
    f32 = mybir.dt.float32
    bf16 = mybir.dt.bfloat16
    k_dt = bf16 if k_dtype == "bfloat16" else f32
    v_dt = bf16 if v_dtype == "bfloat16" else f32
    mult = mybir.AluOpType.mult
    add = mybir.AluOpType.add
    AX = mybir.AxisListType.X
    Exp = mybir.ActivationFunctionType.Exp

    nc = bacc.Bacc(
        "TRN2",
        target_bir_lowering=False,
        debug=False,
        num_devices=NCORES,
    )

    q_d = nc.dram_tensor("query_in", [B_LOC, H], f32, kind="ExternalInput").ap()
    k_d = nc.dram_tensor("keys_in", [B_LOC, s, H], k_dt, kind="ExternalInput").ap()
    v_d = nc.dram_tensor("values_in", [B_LOC, s, H], v_dt, kind="ExternalInput").ap()
    m_d = nc.dram_tensor("mask_in", [B_LOC, s], f32, kind="ExternalInput").ap()
    w_d = nc.dram_tensor("w_in", [H, H], f32, kind="ExternalInput").ap()
    ctx_d = nc.dram_tensor("context_out", [B_LOC, H], f32, kind="ExternalOutput").ap()
    attn_d = nc.dram_tensor("attn_out", [B_LOC, s], f32, kind="ExternalOutput").ap()
    tq_d = nc.dram_tensor("tq_scratch", [B_LOC, H], f32).ap()

    def bcast_row(row_ap):
        """[N] DRAM row -> [128, N] partition-broadcast access pattern."""
        return bass.AP(
            tensor=row_ap.tensor,
            offset=row_ap.offset,
            ap=[[0, P]] + [list(x) for x in row_ap.ap],
        )

    def fused_dot(out, in0, in1, scale, accum_out):
        """accum_out[p] = sum_f in0[p,f]*in1[p,f]*scale, via one DVE op."""
        nc.vector.scalar_tensor_tensor(
            out=out, in0=in0, scalar=float(scale), in1=in1,
            op0=mult, op1=mult, accum_out=accum_out,
        )

    with tile.TileContext(nc) as tc, ExitStack() as ctx:
        kv = ctx.enter_context(tc.tile_pool(name="kv", bufs=7))
        wp = ctx.enter_context(tc.tile_pool(name="wp", bufs=2))
        sm = ctx.enter_context(tc.tile_pool(name="sm", bufs=2))
        ones = ctx.enter_context(tc.tile_pool(name="ones", bufs=1))
        ps = ctx.enter_context(tc.tile_pool(name="ps", bufs=4, space="PSUM"))
        psb = ctx.enter_context(tc.tile_pool(name="psb", bufs=4, space="PSUM"))

        if preduce == "pe":
            from concourse.masks import make_identity

            ident = ones.tile([P, P], f32, tag="ident")
            make_identity(nc, ident)
            ones_pos = ones.tile([1, P], f32, tag="onesp")
            nc.vector.memset(ones_pos, 1.0)

        def sum_reduce_inv(rowsum, b):
            """[128,1] row sums -> [128,1] broadcast 1/total on all partitions."""
            if preduce == "gpsimd":
                g = sm.tile([P, 1], f32, tag="gsum")
                nc.gpsimd.partition_all_reduce(
                    g, rowsum, channels=P, reduce_op=bass_isa.ReduceOp.add
                )
                inv = sm.tile([P, 1], f32, tag="inv")
                nc.vector.reciprocal(inv, g)
                return inv
            tr = psb.tile([1, P], f32, tag="tr")
            nc.tensor.transpose(tr, rowsum, ident[:, 0:1])
            gsb = sm.tile([1, 1], f32, tag="gsb")
            nc.vector.reduce_sum(out=gsb, in_=tr, axis=AX)
            invs = sm.tile([1, 1], f32, tag="invs")
            nc.vector.reciprocal(invs, gsb)
            bc = psb.tile([P, 1], f32, tag="bc")
            nc.tensor.matmul(bc, lhsT=ones_pos, rhs=invs, start=True, stop=True)
            inv = sm.tile([P, 1], f32, tag="inv")
            nc.vector.tensor_copy(out=inv, in_=bc)
            return inv

        # ---------------- W load (once) + tq for both batches ----------------
        w_view = w_d.rearrange("(p c) h -> p c h", p=P)  # row p*HC+c at part p
        wts = []
        for half in range(2):
            wt = wp.tile([P, HC // 2, H], f32, tag="w")
            nc.sync.dma_start(
                out=wt, in_=w_view[:, half * (HC // 2) : (half + 1) * (HC // 2), :]
            )
            wts.append(wt)
        tq_kvs = []
        for b in range(B_LOC):
            q_bc = sm.tile([P, H], f32, tag="qbc")
            nc.gpsimd.dma_start(out=q_bc, in_=bcast_row(q_d[b]))
            tq_sb = sm.tile([P, HC], f32, tag="tqsb")
            prod_w = sm.tile([P, H], f32, tag="prodw")
            for half in range(2):
                for j in range(HC // 2):
                    c = half * (HC // 2) + j
                    fused_dot(prod_w, wts[half][:, j, :], q_bc, 1.0, tq_sb[:, c : c + 1])
            # partition-major [128, 8] -> flat [1024] -> broadcast [128, 1024]
            nc.sync.dma_start(out=tq_d[b].rearrange("(p c) -> p c", p=P), in_=tq_sb)
            tq_bc = sm.tile([P, H], f32, tag="tqbc")
            nc.gpsimd.dma_start(out=tq_bc, in_=bcast_row(tq_d[b]))
            if k_dt != f32:
                tq_kv = sm.tile([P, H], k_dt, tag="tqkv")
                nc.vector.tensor_copy(out=tq_kv, in_=tq_bc)
            else:
                tq_kv = tq_bc
            tq_kvs.append(tq_kv)

        for b in range(B_LOC):
            tq_kv = tq_kvs[b]
            k_view = k_d[b].rearrange("(p i) h -> p i h", p=P)
            v_view = v_d[b].rearrange("(p i) h -> p i h", p=P)

            # mask term: (mask-1)*1e9  (0 where kept, -1e9 where masked)
            mask_f = sm.tile([P, ni], f32, tag="maskf")
            nc.sync.dma_start(out=mask_f, in_=m_d[b].rearrange("(p i) -> p i", p=P))
            mterm = sm.tile([P, ni], f32, tag="mterm")
            nc.vector.tensor_scalar(
                out=mterm, in0=mask_f, scalar1=BIG, scalar2=-BIG, op0=mult, op1=add
            )

            scores = sm.tile([P, ni], f32, tag="scores")
            sc_m = sm.tile([P, ni], f32, tag="scm")
            p_t = sm.tile([P, ni], f32, tag="pt")
            if v_dt != f32:
                p_mm = sm.tile([P, ni], v_dt, tag="pmm")
            else:
                p_mm = p_t
            prod_kv = sm.tile([P, H], k_dt, tag="prodkv")
            ps_lo = ps.tile([1, 512], f32, tag="ps")
            ps_hi = ps.tile([1, 512], f32, tag="ps")

            for t in range(nt):
                cols = slice(t * t_i, (t + 1) * t_i)
                kt = kv.tile([P, t_i, H], k_dt, tag="kv")
                nc.sync.dma_start(out=kt, in_=k_view[:, cols, :])
                vt = kv.tile([P, t_i, H], v_dt, tag="kv")
                nc.scalar.dma_start(out=vt, in_=v_view[:, cols, :])
                for j in range(t_i):
                    i_abs = t * t_i + j
                    fused_dot(
                        prod_kv, kt[:, j, :], tq_kv, INV_SCALE,
                        scores[:, i_abs : i_abs + 1],
                    )
                # masked logits -> unnormalized softmax numerator for the tile
                nc.vector.tensor_add(sc_m[:, cols], scores[:, cols], mterm[:, cols])
                nc.scalar.activation(
                    out=p_t[:, cols], in_=sc_m[:, cols], func=Exp, bias=0.0, scale=1.0
                )
                if v_dt != f32:
                    nc.vector.tensor_copy(out=p_mm[:, cols], in_=p_t[:, cols])
                for j in range(t_i):
                    i_abs = t * t_i + j
                    first = i_abs == 0
                    last = i_abs == ni - 1
                    nc.tensor.matmul(
                        ps_lo,
                        lhsT=p_mm[:, i_abs : i_abs + 1],
                        rhs=vt[:, j, 0:512],
                        start=first,
                        stop=last,
                    )
                    nc.tensor.matmul(
                        ps_hi,
                        lhsT=p_mm[:, i_abs : i_abs + 1],
                        rhs=vt[:, j, 512:1024],
                        start=first,
                        stop=last,
                    )

            # ---------------- normalization ----------------
            rowsum = sm.tile([P, 1], f32, tag="rowsum")
            nc.vector.reduce_sum(out=rowsum, in_=p_t, axis=AX)
            inv = sum_reduce_inv(rowsum, b)
            attn_f = sm.tile([P, ni], f32, tag="attnf")
            nc.vector.tensor_scalar_mul(attn_f, p_t, inv)
            nc.sync.dma_start(
                out=attn_d[b].rearrange("(p i) -> p i", p=P), in_=attn_f
            )
            ctx_sb = sm.tile([1, H], f32, tag="ctxsb")
            nc.vector.tensor_copy(out=ctx_sb[0:1, 0:512], in_=ps_lo)
            nc.vector.tensor_copy(out=ctx_sb[0:1, 512:1024], in_=ps_hi)
            ctx_fin = sm.tile([1, H], f32, tag="ctxfin")
            nc.vector.tensor_scalar_mul(ctx_fin, ctx_sb, inv[0:1, :])
            nc.sync.dma_start(out=ctx_d[b], in_=ctx_fin[0:1, :])

    nc.compile()
    return nc


_CACHE = {}


def _get_nc():
    key = (S, K_DTYPE, V_DTYPE, T_I, PREDUCE)
    if key not in _CACHE:
        _CACHE[key] = build()
    return _CACHE[key]


def kernel(query, keys, values, mask, W_attn):
    _, _, _, _, _, run_bass_kernel_spmd = _mods()
    import ml_dtypes

    k_np = ml_dtypes.bfloat16 if K_DTYPE == "bfloat16" else np.float32
    v_np = ml_dtypes.bfloat16 if V_DTYPE == "bfloat16" else np.float32

    query = np.asarray(query, dtype=np.float32)
    keys = np.asarray(keys)
    values = np.asarray(values)
    mask_f = np.asarray(mask, dtype=np.float32)
    W_attn = np.asarray(W_attn, dtype=np.float32)

    nc = _get_nc()
    in_maps = []
    for m in range(NCORES):
        sl = slice(B_LOC * m, B_LOC * (m + 1))
        in_maps.append(
            {
                "query_in": np.ascontiguousarray(query[sl]),
                "keys_in": np.ascontiguousarray(keys[sl]).astype(k_np),
                "values_in": np.ascontiguousarray(values[sl]).astype(v_np),
                "mask_in": np.ascontiguousarray(mask_f[sl]),
                "w_in": W_attn,
            }
        )
    res = run_bass_kernel_spmd(nc, in_maps, list(range(NCORES))).results
    context = np.concatenate([r["context_out"] for r in res], axis=0)
    attn = np.concatenate([r["attn_out"] for r in res], axis=0)
    return context, attn
